# revision 9
# baseline (speedup 1.0000x reference)
"""Trainium2 Bass kernel for CrossAttention (B=2, C=128, H=W=64, heads=4, d=64).

Sharding: one (batch, head) pair per NeuronCore (2*4 = 8 cores).

Per-core computation (all on device):
  q2 = [Wq|Wq] @ x_b          [128, 4096]  (q duplicated on both partition
                                            halves so QK matmuls can be
                                            row-tiled across PE halves)
  k2 = [Wk|Wk] @ y_b          [128, 4096]  (same duplication)
  vT = (y_b.T @ WvT_h)        [4096, 65]   (j on partitions; ones column at
                                            m=64 accumulates the softmax
                                            denominator l[i])
  row-l2-normalize q, k along the 4096 axis (both norms folded into k as
        g[d] = 1/(||q_d|| * ||k_d||))
  sim_T[j, i] = sum_d k[d, j] q[d, i]  emitted as PAIRS of j-tiles: the even
        j-tile runs on PE row-groups 0-1 (tile_position (0,0)), the odd
        j-tile concurrently on row-groups 2-3 (tile_position (64,0)) -- the
        d=64 contraction only fills half the array, so two independent
        matmuls share it for ~2x QK throughput.
  e_T = exp(SCALE * sim_T), one [128, 1024] tile per j-pair, ALTERNATING
        engines: ACT runs the exact spline exp; DVE runs a Schraudolph
        bit-trick exp (z = sim*2^23*log2e*SCALE + (127*2^23 - 60801.48),
        converted to int32 and reinterpreted as float32). The Schraudolph
        approximation has ~3% per-element error but the softmax ratio
        cancels nearly all of it (validated: ~4e-3 final rel err).
  outT_aug[m, i] = sum_j vT_aug[j, m] * e_T[j, i]   (accumulated in PSUM)
  po = WoT_h.T @ outT[0:64]   [128, 4096]  (projection; NOT divided by l)
Host: out[b] = sum_h po[b, h] / l[b, h] + bias; reshape to [2, 128, 64, 64].
(The division by the softmax denominator commutes with the out-projection's
channel contraction, so it is applied on the host per head.)

Logits are bounded (|sim| <= 1 by Cauchy-Schwarz, ~0.14 in practice), so the
softmax max-subtraction is skipped and Schraudolph never over/underflows.

The attention matmuls run in float32r (full PE rate; ~11-bit mantissa).
"""

import numpy as np
import ml_dtypes

import concourse.bacc as bacc
import concourse.mybir as mybir
from concourse.bass import ts, ds
from concourse.tile import TileContext
from concourse.bass_utils import run_bass_kernel_spmd

F32 = mybir.dt.float32
F32R = mybir.dt.float32r
BF16 = mybir.dt.bfloat16
I16 = mybir.dt.int16

B, C, HW = 2, 128, 4096
HEADS, D = 4, 64
HIDDEN = HEADS * D
SCALE = 10.0
N_CORES = 8

IC = 512             # i-axis chunk per pv accumulation
N_IC = HW // IC      # 8
N_J = HW // 128      # 32 j-tiles of 128
N_P = N_J // 2       # 16 j-pairs per i-chunk
TOT = N_IC * N_P     # 128 global pairs
NCH = HW // 512      # 8 projection chunks

LOG2E = 1.4426950408889634
# bf16 Schraudolph constants (the e_T tiles are bf16 so the DVE bit-trick
# writes int16: bf16 has a 7-bit mantissa -> 2^7 units per octave)
EXP_A = SCALE * LOG2E * 128.0              # fold exp scale into Schraudolph mult
EXP_B = 127.0 * 128.0 - 60801.48 / 65536.0  # Schraudolph bias (min max-rel-err)

# exp engine schedule: strict ACT/DVE alternation. A skewed split (e.g.
# 17/32) is slightly better in per-engine arithmetic but measurably worse on
# hardware: the occasional back-to-back same-engine pairs cascade stalls.
ACT_FRAC_NUM, ACT_FRAC_DEN = 16, 32


def _use_act(g):
    return (g * ACT_FRAC_NUM) % ACT_FRAC_DEN < ACT_FRAC_NUM


def _emit_one(nc, tc, io, mm_fast, rep, token=None):
    """Emit one full forward pass. rep distinguishes pool names across repeats.

    token: optional [128, 1] tile threading a serializing dependency between
    repeats (rep r+1's input tiles are seeded from the token written at the
    end of rep r), so the repeat-diff timing measures true serial latency.
    """
    xb, yb, wq2T, wk2T, wvT, woT, outp, lout = io
    MMDT = F32R if mm_fast else F32
    Exp = mybir.ActivationFunctionType.Exp
    Square = mybir.ActivationFunctionType.Square
    mult = mybir.AluOpType.mult

    with tc.tile_pool(name=f"big{rep}", bufs=1) as big, \
         tc.tile_pool(name=f"const{rep}", bufs=1) as const:
        wq_sb = const.tile([C, 2 * D], BF16)
        wk_sb = const.tile([C, 2 * D], BF16)
        wv_sb = const.tile([C, D], BF16)
        wo_sb = const.tile([D, C], F32)
        wo_r = const.tile([D, C], MMDT)
        x_r = big.tile([C, HW], BF16)
        y_r = big.tile([C, HW], BF16)
        if token is not None and rep > 0:
            # seed every input tile with the previous rep's token: the input
            # DMAs then carry a WAW dependency on it, serializing the reps
            for tile in (wq_sb, wk_sb, wv_sb, x_r, y_r):
                nc.vector.tensor_copy(tile[:, 0:1], token)
            nc.vector.tensor_copy(wo_sb[:, 0:1], token[0:D])
        # weights first: tiny DMAs that gate the first projection matmuls
        nc.sync.dma_start(wq_sb, wq2T[:])
        nc.sync.dma_start(wk_sb, wk2T[:])
        nc.sync.dma_start(wv_sb, wvT[:])
        nc.sync.dma_start(wo_sb, woT[:])

        # chunked input DMA so projections start before the full load lands;
        # xb on the SP HWDGE queue, yb on the ACT HWDGE queue (parallel)
        for t in range(NCH):
            nc.sync.dma_start(x_r[:, ts(t, 512)], xb[:, ts(t, 512)])
            nc.scalar.dma_start(y_r[:, ts(t, 512)], yb[:, ts(t, 512)])
        q2_sb = big.tile([C, HW], MMDT)      # duplicated q (both halves)
        k2_sb = big.tile([C, HW], F32)       # duplicated k, pre-norm
        k2_r = big.tile([C, HW], MMDT)       # duplicated k, norm folded
        vT_sb = big.tile([128, N_J, D + 1], BF16)
        qparts = const.tile([C, NCH], F32)
        kparts = const.tile([C, NCH], F32)
        ones_sb = const.tile([128, 1], F32)
        nc.vector.memset(ones_sb, 1.0)

        # ---------------- Stage A: projections + normalization ----------
        with tc.tile_pool(name=f"psA{rep}", bufs=2, space="PSUM") as psA:
            # PE warm-up: dense dummy matmuls during the DMA window nudge the
            # HAM clock gate toward 2.4 GHz before the real work lands.
            warm = const.tile([128, 512], F32)
            nc.vector.memset(warm, 0.0)
            for w in range(2):
                pw = psA.tile([128, 512], F32, tag="pw")
                nc.tensor.matmul(pw, lhsT=warm[:, 0:128], rhs=warm[:],
                                 start=True, stop=True)
            nc.vector.tensor_copy(wo_r[:], wo_sb[:])
            for t in range(NCH):
                pq = psA.tile([C, 512], F32, tag="pq")
                nc.tensor.matmul(pq, lhsT=wq_sb[:], rhs=x_r[:, ts(t, 512)],
                                 start=True, stop=True)
                # squares first so the norm-reduction chain unblocks early;
                # q copies alternate ACT/DVE to balance stage-A engine load
                scr_ps = psA.tile([128, 512], F32, tag="pw")
                nc.scalar.activation(scr_ps, pq, Square,
                                     accum_out=qparts[:, t:t + 1])
                if t % 2 == 0:
                    nc.scalar.copy(q2_sb[:, ts(t, 512)], pq)
                else:
                    nc.vector.tensor_copy(q2_sb[:, ts(t, 512)], pq)
                pk = psA.tile([C, 512], F32, tag="pk")
                nc.tensor.matmul(pk, lhsT=wk_sb[:], rhs=y_r[:, ts(t, 512)],
                                 start=True, stop=True)
                scr_ps2 = psA.tile([128, 512], F32, tag="pw")
                nc.scalar.activation(scr_ps2, pk, Square,
                                     accum_out=kparts[:, t:t + 1])
                nc.vector.tensor_copy(k2_sb[:, ts(t, 512)], pk)
                # vT blocks for this yb chunk: (yb 128-col).T @ wv -> [128, 64]
                # 4 blocks share one PSUM tile so a single DVE copy moves them
                pv = psA.tile([128, 4, D], F32, tag="pv")
                for jj in range(4):
                    nc.tensor.matmul(pv[:, jj, :],
                                     lhsT=y_r[:, ts(4 * t + jj, 128)],
                                     rhs=wv_sb[:], start=True, stop=True)
                nc.vector.tensor_copy(vT_sb[:, 4 * t:4 * t + 4, 0:D], pv)
            nc.vector.tensor_copy(vT_sb[:, :, D],
                                  ones_sb.to_broadcast((128, N_J)))

        # g[d] = 1/sqrt(ssq_q[d] * ssq_k[d]), computed entirely on DVE so the
        # ACT table set (exp_and_others, loaded once for the Squares) never
        # switches mid-kernel: bit-trick rsqrt seed + 3 Newton iterations.
        # All at [128, 1]: the duplicated projection rows give a duplicated
        # g for free.
        ssq_q = const.tile([C, 1], F32)
        ssq_k = const.tile([C, 1], F32)
        nc.vector.reduce_sum(ssq_q, qparts[:], axis=mybir.AxisListType.X)
        nc.vector.reduce_sum(ssq_k, kparts[:], axis=mybir.AxisListType.X)
        P = const.tile([C, 1], F32)
        nc.vector.tensor_mul(P, ssq_q, ssq_k)
        nc.vector.tensor_scalar_max(P, P, 1e-24)
        hi = const.tile([C, 1], mybir.dt.int32)
        # 0x5f3759df - h == ((h >> 1) ^ 0xffffffff) + 0x5f3759e0, fused
        nc.vector.tensor_scalar(hi, P.bitcast(mybir.dt.int32), 1, -1,
                                op0=mybir.AluOpType.arith_shift_right,
                                op1=mybir.AluOpType.bitwise_xor)
        nc.vector.tensor_scalar(hi, hi, 0x5F3759E0, None,
                                op0=mybir.AluOpType.add)
        g = const.tile([C, 1], F32)
        gt = const.tile([C, 1], F32)
        yv = hi.bitcast(F32)
        for it in range(2):
            src = yv if it == 0 else g
            nc.vector.tensor_mul(gt, src, src)                       # y^2
            nc.vector.scalar_tensor_tensor(gt, gt, -0.5, P,
                                           op0=mult, op1=mult)       # -.5Py^2
            nc.vector.scalar_tensor_tensor(g, gt, 1.5, src,
                                           op0=mybir.AluOpType.add,
                                           op1=mult)                 # refined
        # fold both norms into k; tiny first chunk so QK(0) unblocks early
        nc.vector.tensor_scalar(k2_r[:, 0:256], k2_sb[:, 0:256],
                                g, None, op0=mult)
        nc.vector.tensor_scalar(k2_r[:, 256:1024], k2_sb[:, 256:1024],
                                g, None, op0=mult)
        for c4 in range(1, 4):
            nc.vector.tensor_scalar(k2_r[:, ts(c4, 1024)], k2_sb[:, ts(c4, 1024)],
                                    g, None, op0=mult)

        # ------- Main loop: attention in j-pairs + fused epilogue ---------
        # Per global pair g = 16*ic + p covering j-tiles (2p, 2p+1) on
        # i-chunk ic: QK row-tiled pair -> one exp tile [128, 1024]
        # (ACT or DVE per schedule) -> 2 accumulating PV matmuls.
        with tc.tile_pool(name=f"qkps{rep}", bufs=3, space="PSUM") as qkps_pool, \
             tc.tile_pool(name=f"pvps{rep}", bufs=1, space="PSUM") as pvps_pool, \
             tc.tile_pool(name=f"pops{rep}", bufs=1, space="PSUM") as pops_pool, \
             tc.tile_pool(name=f"eta{rep}", bufs=5) as ea_pool, \
             tc.tile_pool(name=f"etd{rep}", bufs=5) as ed_pool, \
             tc.tile_pool(name=f"ot{rep}", bufs=2) as ot_pool:
            LOOKAHEAD = 2
            qk_tiles = {}
            e_tiles = {}
            pv_tiles = {}

            def emit_qk(g):
                ic, p = divmod(g, N_P)
                qk = qkps_pool.tile([128, 2 * IC], F32, tag="qk")
                oc = ds(ic * IC, IC)
                nc.tensor.matmul(qk[:, 0:IC],
                                 lhsT=k2_r[0:D, ts(2 * p, 128)],
                                 rhs=q2_sb[0:D, oc],
                                 start=True, stop=True,
                                 tile_position=(0, 0))
                nc.tensor.matmul(qk[:, IC:2 * IC],
                                 lhsT=k2_r[D:2 * D, ts(2 * p + 1, 128)],
                                 rhs=q2_sb[D:2 * D, oc],
                                 start=True, stop=True,
                                 tile_position=(64, 0))
                qk_tiles[g] = qk

            def emit_exp(g):
                qk = qk_tiles.pop(g)
                # Schraudolph bit-trick exp on BOTH engines (ACT's spline Exp
                # with bf16 output measures 1587ns/tile vs 824ns for the
                # Copy+scale+bias → int16 bit trick; DVE is 856ns).  The
                # smooth per-element error (~1%) cancels in the softmax
                # ratio: all-Schraudolph rel err 4.5e-3 vs 4.3e-3 mixed.
                if _use_act(g):
                    eT = ea_pool.tile([128, 2 * IC], BF16, tag="eT")
                    nc.scalar.activation(eT.bitcast(I16), qk,
                                         mybir.ActivationFunctionType.Copy,
                                         bias=EXP_B, scale=EXP_A)
                else:
                    eT = ed_pool.tile([128, 2 * IC], BF16, tag="eTd")
                    nc.vector.tensor_scalar(eT.bitcast(I16), qk, EXP_A, EXP_B,
                                            op0=mult, op1=mybir.AluOpType.add)
                e_tiles[g] = eT

            def emit_pv(g):
                ic, p = divmod(g, N_P)
                if p == 0:
                    pv_ps = pvps_pool.tile([D + 1, IC], F32, tag="pv")
                    pv_tiles[ic] = pv_ps
                eT = e_tiles.pop(g)
                nc.tensor.matmul(pv_tiles[ic][:], lhsT=vT_sb[:, 2 * p, :],
                                 rhs=eT[:, 0:IC],
                                 start=(p == 0), stop=False)
                nc.tensor.matmul(pv_tiles[ic][:], lhsT=vT_sb[:, 2 * p + 1, :],
                                 rhs=eT[:, IC:2 * IC],
                                 start=False, stop=(p == N_P - 1))

            def epilogue(ic):
                pv_ps = pv_tiles.pop(ic)
                oc = ds(ic * IC, IC)
                outT = ot_pool.tile([D + 1, IC], MMDT, tag="outT")
                # split the pv evacuation across both elementwise engines so
                # whichever frees first starts; pv_ps unblocks ~2x sooner
                nc.scalar.copy(outT[:, 0:IC // 2], pv_ps[:, 0:IC // 2])
                nc.vector.tensor_copy(outT[:, IC // 2:IC], pv_ps[:, IC // 2:IC])
                po = pops_pool.tile([C, IC], F32, tag="po")
                nc.tensor.matmul(po, lhsT=wo_r[:], rhs=outT[0:D, :],
                                 start=True, stop=True)
                out_sb = ot_pool.tile([C, IC], BF16, tag="out_sb")
                nc.vector.tensor_copy(out_sb, po)
                nc.sync.dma_start(outp[:, oc], out_sb)
                nc.sync.dma_start(lout[:, oc], outT[D:D + 1, :].bitcast(F32))
                if token is not None and ic == N_IC - 1:
                    nc.vector.tensor_copy(token, out_sb[:, 0:1])

            for g in range(LOOKAHEAD):
                emit_qk(g)
            for g in range(TOT):
                if g + LOOKAHEAD < TOT:
                    emit_qk(g + LOOKAHEAD)
                emit_exp(g)
                emit_pv(g)
                if g % N_P == N_P - 1:
                    epilogue(g // N_P)


def build_nc(mm_fast=True, repeat=1, probe=False):
    nc = bacc.Bacc(None, target_bir_lowering=False)
    xb = nc.dram_tensor("xb", [C, HW], BF16, kind="ExternalInput")
    yb = nc.dram_tensor("yb", [C, HW], BF16, kind="ExternalInput")
    wq2T = nc.dram_tensor("wq2T", [C, 2 * D], BF16, kind="ExternalInput")
    wk2T = nc.dram_tensor("wk2T", [C, 2 * D], BF16, kind="ExternalInput")
    wvT = nc.dram_tensor("wvT", [C, D], BF16, kind="ExternalInput")
    woT = nc.dram_tensor("woT", [D, C], F32, kind="ExternalInput")
    outp = nc.dram_tensor("outp", [C, HW], BF16, kind="ExternalOutput")
    lout = nc.dram_tensor("lout", [1, HW], F32, kind="ExternalOutput")
    io = (xb, yb, wq2T, wk2T, wvT, woT, outp, lout)
    with TileContext(nc) as tc:
        with tc.tile_pool(name="tok", bufs=1) as tokp:
            token = (tokp.tile([C, 1], F32, name="token")
                     if repeat > 1 else None)
            for rep in range(repeat):
                _emit_one(nc, tc, io, mm_fast, rep, token=token)
    nc.finalize()
    return nc


_NC_CACHE = {}


def _get_nc(mm_fast=True, repeat=1):
    key = (mm_fast, repeat)
    if key not in _NC_CACHE:
        _NC_CACHE[key] = build_nc(mm_fast, repeat)
    return _NC_CACHE[key]


def make_in_maps(x, y, W_qkv, W_out):
    x = np.asarray(x, np.float32).reshape(B, C, HW)
    y = np.asarray(y, np.float32).reshape(B, C, HW)
    W_qkv = np.asarray(W_qkv, np.float32)
    W_out = np.asarray(W_out, np.float32)
    in_maps = []
    for core in range(N_CORES):
        b, h = core // HEADS, core % HEADS
        sl = slice(h * D, (h + 1) * D)
        wqT = W_qkv[sl, :].T                                     # [C, D]
        wkT = W_qkv[HIDDEN + h * D:HIDDEN + (h + 1) * D, :].T
        wvT = W_qkv[2 * HIDDEN + h * D:2 * HIDDEN + (h + 1) * D, :].T
        bf = ml_dtypes.bfloat16
        in_maps.append({
            "xb": np.ascontiguousarray(x[b].astype(bf)),
            "yb": np.ascontiguousarray(y[b].astype(bf)),
            "wq2T": np.ascontiguousarray(
                np.concatenate([wqT, wqT], axis=1).astype(bf)),
            "wk2T": np.ascontiguousarray(
                np.concatenate([wkT, wkT], axis=1).astype(bf)),
            "wvT": np.ascontiguousarray(wvT.astype(bf)),
            "woT": np.ascontiguousarray(W_out[:, sl].T),
        })
    return in_maps


def gather(results, b_out):
    b_out = np.asarray(b_out, np.float32)
    out = np.zeros((B, C, HW), np.float32)
    for core in range(N_CORES):
        # per-head softmax-denominator division, folded to the host (it
        # commutes with the out-projection's channel contraction)
        out[core // HEADS] += (results[core]["outp"].astype(np.float32)
                               / results[core]["lout"])
    out += b_out[None, :, None]
    return out.reshape(B, C, 64, 64)


def kernel(x, y, W_qkv, W_out, b_out):
    nc = _get_nc(mm_fast=True)
    in_maps = make_in_maps(x, y, W_qkv, W_out)
    res = run_bass_kernel_spmd(nc, in_maps, core_ids=list(range(N_CORES)))
    return gather(res.results, b_out)



# revision 10
# speedup vs baseline: 2.8450x; 2.8450x over previous
"""Trainium2 Bass kernel for CrossAttention (B=2, C=128, H=W=64, heads=4, d=64).

Sharding: one (batch, head) pair per NeuronCore (2*4 = 8 cores).

v3 vs v2 (HW-ablation driven):
  - exp tiles are SPLIT: ACT computes columns 0:512, DVE 512:1024 of every
    [128,1024] tile concurrently (both Schraudolph), halving the
    qk->exp->pv latency chain that serialized the v2 pipeline (ablations:
    qk-only 64us, +exp 116us, +pv 172us -- exp/pv added nearly their full
    engine time as WALL time because of the 3-buffer qk WAR chain).
  - QK operands are bf16 (q2/k2_r): bf16 moving streams measured 300ns vs
    396ns fp32r at N=512; precision impact ~5e-5 on logits.
  - outT/wo_r are bf16 (po matmul joins the fast path); the softmax
    denominator row goes to a separate f32 tile for the lout DMA.
"""

import os
import numpy as np
import ml_dtypes

import concourse.bacc as bacc
import concourse.mybir as mybir
from concourse.bass import ts, ds
from concourse.tile import TileContext
from concourse.bass_utils import run_bass_kernel_spmd

F32 = mybir.dt.float32
F32R = mybir.dt.float32r
BF16 = mybir.dt.bfloat16
I16 = mybir.dt.int16

B, C, HW = 2, 128, 4096
HEADS, D = 4, 64
HIDDEN = HEADS * D
SCALE = 10.0
N_CORES = 8

IC = 512             # i-axis chunk per pv accumulation
N_IC = HW // IC      # 8
N_J = HW // 128      # 32 j-tiles of 128
N_P = N_J // 2       # 16 j-pairs per i-chunk
TOT = N_IC * N_P     # 128 global pairs
NCH = HW // 512      # 8 projection chunks

LOOKAHEAD = 2
QK_BUFS = 3

LOG2E = 1.4426950408889634
EXP_A = SCALE * LOG2E * 128.0              # fold exp scale into Schraudolph mult
EXP_B = 127.0 * 128.0 - 60801.48 / 65536.0  # Schraudolph bias (min max-rel-err)


def _use_act(g):
    return g % 2 == 0


def _emit_one(nc, tc, io, mm_fast, rep, token=None):
    """Emit one full forward pass. rep distinguishes pool names across repeats."""
    xb, yb, wq2T, wk2T, wvT, woT, outp, lout = io
    MMDT = F32R if mm_fast else F32
    Square = mybir.ActivationFunctionType.Square
    Copy = mybir.ActivationFunctionType.Copy
    mult = mybir.AluOpType.mult
    add = mybir.AluOpType.add

    with tc.tile_pool(name=f"big{rep}", bufs=1) as big, \
         tc.tile_pool(name=f"const{rep}", bufs=1) as const:
        wq_sb = const.tile([C, 2 * D], BF16)
        wk_sb = const.tile([C, 2 * D], BF16)
        wv_sb = const.tile([C, D], BF16)
        wo_sb = const.tile([D, C], F32)
        wo_r = const.tile([D, C], BF16)
        x_r = big.tile([C, HW], BF16)
        y_r = big.tile([C, HW], BF16)
        if token is not None and rep > 0:
            # seed every input tile with the previous rep's token: the input
            # DMAs then carry a WAW dependency on it, serializing the reps
            for tile in (wq_sb, wk_sb, wv_sb, x_r, y_r):
                nc.vector.tensor_copy(tile[:, 0:1], token)
            nc.vector.tensor_copy(wo_sb[:, 0:1], token[0:D])
        # weights first: tiny DMAs that gate the first projection matmuls
        nc.sync.dma_start(wq_sb, wq2T[:])
        nc.sync.dma_start(wk_sb, wk2T[:])
        nc.sync.dma_start(wv_sb, wvT[:])
        nc.sync.dma_start(wo_sb, woT[:])

        # chunked input DMA so projections start before the full load lands;
        # xb on the SP HWDGE queue, yb on the ACT HWDGE queue (parallel)
        for t in range(NCH):
            nc.sync.dma_start(x_r[:, ts(t, 512)], xb[:, ts(t, 512)])
            nc.scalar.dma_start(y_r[:, ts(t, 512)], yb[:, ts(t, 512)])
        q2_sb = big.tile([C, HW], BF16)      # duplicated q (both halves)
        k2_sb = big.tile([C, HW], F32)       # duplicated k, pre-norm
        k2_r = big.tile([C, HW], BF16)       # duplicated k, norm folded
        vT_sb = big.tile([128, N_J, D + 1], BF16)
        qparts = const.tile([C, NCH], F32)
        kparts = const.tile([C, NCH], F32)
        ones_sb = const.tile([128, 1], F32)
        nc.vector.memset(ones_sb, 1.0)

        # ---------------- Stage A: projections + normalization ----------
        with tc.tile_pool(name=f"psA{rep}", bufs=2, space="PSUM") as psA:
            # PE warm-up: dense dummy matmuls during the DMA window nudge the
            # HAM clock gate toward 2.4 GHz before the real work lands.
            warm = const.tile([128, 512], F32)
            nc.vector.memset(warm, 0.0)
            for w in range(2):
                pw = psA.tile([128, 512], F32, tag="pw")
                nc.tensor.matmul(pw, lhsT=warm[:, 0:128], rhs=warm[:],
                                 start=True, stop=True)
            nc.vector.tensor_copy(wo_r[:], wo_sb[:])
            for t in range(NCH):
                pq = psA.tile([C, 512], F32, tag="pq")
                nc.tensor.matmul(pq, lhsT=wq_sb[:], rhs=x_r[:, ts(t, 512)],
                                 start=True, stop=True)
                # squares first so the norm-reduction chain unblocks early;
                # q copies alternate ACT/DVE to balance stage-A engine load
                scr_ps = psA.tile([128, 512], F32, tag="pw")
                nc.scalar.activation(scr_ps, pq, Square,
                                     accum_out=qparts[:, t:t + 1])
                if t % 2 == 0:
                    nc.scalar.copy(q2_sb[:, ts(t, 512)], pq)
                else:
                    nc.vector.tensor_copy(q2_sb[:, ts(t, 512)], pq)
                pk = psA.tile([C, 512], F32, tag="pk")
                nc.tensor.matmul(pk, lhsT=wk_sb[:], rhs=y_r[:, ts(t, 512)],
                                 start=True, stop=True)
                scr_ps2 = psA.tile([128, 512], F32, tag="pw")
                nc.scalar.activation(scr_ps2, pk, Square,
                                     accum_out=kparts[:, t:t + 1])
                nc.vector.tensor_copy(k2_sb[:, ts(t, 512)], pk)
                # vT blocks for this yb chunk: (yb 128-col).T @ wv -> [128, 64]
                # 4 blocks share one PSUM tile so a single DVE copy moves them
                pv = psA.tile([128, 4, D], F32, tag="pv")
                for jj in range(4):
                    nc.tensor.matmul(pv[:, jj, :],
                                     lhsT=y_r[:, ts(4 * t + jj, 128)],
                                     rhs=wv_sb[:], start=True, stop=True)
                nc.vector.tensor_copy(vT_sb[:, 4 * t:4 * t + 4, 0:D], pv)
            nc.vector.tensor_copy(vT_sb[:, :, D],
                                  ones_sb.to_broadcast((128, N_J)))

        # g[d] = 1/sqrt(ssq_q[d] * ssq_k[d]) on DVE: bit-trick rsqrt seed +
        # 2 Newton iterations, all at [128, 1] (duplicated rows give a
        # duplicated g for free).
        ssq_q = const.tile([C, 1], F32)
        ssq_k = const.tile([C, 1], F32)
        nc.vector.reduce_sum(ssq_q, qparts[:], axis=mybir.AxisListType.X)
        nc.vector.reduce_sum(ssq_k, kparts[:], axis=mybir.AxisListType.X)
        P = const.tile([C, 1], F32)
        nc.vector.tensor_mul(P, ssq_q, ssq_k)
        nc.vector.tensor_scalar_max(P, P, 1e-24)
        hi = const.tile([C, 1], mybir.dt.int32)
        # 0x5f3759df - h == ((h >> 1) ^ 0xffffffff) + 0x5f3759e0, fused
        nc.vector.tensor_scalar(hi, P.bitcast(mybir.dt.int32), 1, -1,
                                op0=mybir.AluOpType.arith_shift_right,
                                op1=mybir.AluOpType.bitwise_xor)
        nc.vector.tensor_scalar(hi, hi, 0x5F3759E0, None,
                                op0=mybir.AluOpType.add)
        g = const.tile([C, 1], F32)
        gt = const.tile([C, 1], F32)
        yv = hi.bitcast(F32)
        for it in range(2):
            src = yv if it == 0 else g
            nc.vector.tensor_mul(gt, src, src)                       # y^2
            nc.vector.scalar_tensor_tensor(gt, gt, -0.5, P,
                                           op0=mult, op1=mult)       # -.5Py^2
            nc.vector.scalar_tensor_tensor(g, gt, 1.5, src,
                                           op0=mybir.AluOpType.add,
                                           op1=mult)                 # refined
        # fold both norms into k; tiny first chunk so QK(0) unblocks early
        nc.vector.tensor_scalar(k2_r[:, 0:256], k2_sb[:, 0:256],
                                g, None, op0=mult)
        nc.vector.tensor_scalar(k2_r[:, 256:1024], k2_sb[:, 256:1024],
                                g, None, op0=mult)
        for c4 in range(1, 4):
            nc.vector.tensor_scalar(k2_r[:, ts(c4, 1024)], k2_sb[:, ts(c4, 1024)],
                                    g, None, op0=mult)

        # ------- Main loop: attention in j-pairs + fused epilogue ---------
        # Per global pair g = 16*ic + p covering j-tiles (2p, 2p+1) on
        # i-chunk ic (256 wide): QK row-tiled pair -> one exp tile
        # [128, 512] (ACT or DVE Schraudolph) -> 2 accumulating PV matmuls.
        with tc.tile_pool(name=f"qkps{rep}", bufs=QK_BUFS, space="PSUM") as qkps_pool, \
             tc.tile_pool(name=f"pvps{rep}", bufs=1, space="PSUM") as pvps_pool, \
             tc.tile_pool(name=f"pops{rep}", bufs=1, space="PSUM") as pops_pool, \
             tc.tile_pool(name=f"eta{rep}", bufs=5) as ea_pool, \
             tc.tile_pool(name=f"ot{rep}", bufs=2) as ot_pool:
            qk_tiles = {}
            e_tiles = {}
            pv_tiles = {}

            def emit_qk(g):
                ic, p = divmod(g, N_P)
                qk = qkps_pool.tile([128, 2 * IC], F32, tag="qk")
                oc = ds(ic * IC, IC)
                nc.tensor.matmul(qk[:, 0:IC],
                                 lhsT=k2_r[0:D, ts(2 * p, 128)],
                                 rhs=q2_sb[0:D, oc],
                                 start=True, stop=True,
                                 tile_position=(0, 0))
                nc.tensor.matmul(qk[:, IC:2 * IC],
                                 lhsT=k2_r[D:2 * D, ts(2 * p + 1, 128)],
                                 rhs=q2_sb[D:2 * D, oc],
                                 start=True, stop=True,
                                 tile_position=(64, 0))
                qk_tiles[g] = qk

            def emit_exp(g):
                qk = qk_tiles.pop(g)
                # Schraudolph bit-trick exp, SPLIT: ACT does the first half
                # (feeds pv mm1), DVE the second, concurrently (~480ns each
                # vs ~840 for a whole tile on one engine).
                eT = ea_pool.tile([128, 2 * IC], BF16, tag="eT")
                nc.scalar.activation(eT.bitcast(I16)[:, 0:IC], qk[:, 0:IC],
                                     Copy, bias=EXP_B, scale=EXP_A)
                nc.vector.tensor_scalar(eT.bitcast(I16)[:, IC:2 * IC],
                                        qk[:, IC:2 * IC], EXP_A, EXP_B,
                                        op0=mult, op1=add)
                e_tiles[g] = eT

            def emit_pv(g):
                ic, p = divmod(g, N_P)
                if p == 0:
                    pv_ps = pvps_pool.tile([D + 1, IC], F32, tag="pv")
                    pv_tiles[ic] = pv_ps
                eT = e_tiles.pop(g)
                nc.tensor.matmul(pv_tiles[ic][:], lhsT=vT_sb[:, 2 * p, :],
                                 rhs=eT[:, 0:IC],
                                 start=(p == 0), stop=False)
                nc.tensor.matmul(pv_tiles[ic][:], lhsT=vT_sb[:, 2 * p + 1, :],
                                 rhs=eT[:, IC:2 * IC],
                                 start=False, stop=(p == N_P - 1))

            def epilogue(ic):
                pv_ps = pv_tiles.pop(ic)
                oc = ds(ic * IC, IC)
                outT = ot_pool.tile([D, IC], BF16, tag="outT")
                lT = ot_pool.tile([1, IC], F32, tag="lT")
                # split the pv evacuation across both elementwise engines so
                # whichever frees first starts; pv_ps unblocks ~2x sooner
                nc.scalar.copy(outT[:, 0:IC // 2], pv_ps[0:D, 0:IC // 2])
                nc.vector.tensor_copy(outT[:, IC // 2:IC],
                                      pv_ps[0:D, IC // 2:IC])
                nc.vector.tensor_copy(lT, pv_ps[D:D + 1, :])
                po = pops_pool.tile([C, IC], F32, tag="po")
                nc.tensor.matmul(po, lhsT=wo_r[:], rhs=outT[:],
                                 start=True, stop=True)
                out_sb = ot_pool.tile([C, IC], BF16, tag="out_sb")
                nc.scalar.copy(out_sb[:, 0:IC // 2], po[:, 0:IC // 2])
                nc.vector.tensor_copy(out_sb[:, IC // 2:IC], po[:, IC // 2:IC])
                nc.sync.dma_start(outp[:, oc], out_sb)
                nc.sync.dma_start(lout[:, oc], lT)
                if token is not None and ic == N_IC - 1:
                    nc.vector.tensor_copy(token, out_sb[:, 0:1])

            for g in range(LOOKAHEAD):
                emit_qk(g)
            for g in range(TOT):
                if g + LOOKAHEAD < TOT:
                    emit_qk(g + LOOKAHEAD)
                emit_exp(g)
                emit_pv(g)
                if g % N_P == N_P - 1:
                    epilogue(g // N_P)


def build_nc(mm_fast=True, repeat=1, probe=False):
    nc = bacc.Bacc(None, target_bir_lowering=False)
    xb = nc.dram_tensor("xb", [C, HW], BF16, kind="ExternalInput")
    yb = nc.dram_tensor("yb", [C, HW], BF16, kind="ExternalInput")
    wq2T = nc.dram_tensor("wq2T", [C, 2 * D], BF16, kind="ExternalInput")
    wk2T = nc.dram_tensor("wk2T", [C, 2 * D], BF16, kind="ExternalInput")
    wvT = nc.dram_tensor("wvT", [C, D], BF16, kind="ExternalInput")
    woT = nc.dram_tensor("woT", [D, C], F32, kind="ExternalInput")
    outp = nc.dram_tensor("outp", [C, HW], BF16, kind="ExternalOutput")
    lout = nc.dram_tensor("lout", [1, HW], F32, kind="ExternalOutput")
    io = (xb, yb, wq2T, wk2T, wvT, woT, outp, lout)
    with TileContext(nc) as tc:
        with tc.tile_pool(name="tok", bufs=1) as tokp:
            token = (tokp.tile([C, 1], F32, name="token")
                     if repeat > 1 else None)
            for rep in range(repeat):
                _emit_one(nc, tc, io, mm_fast, rep, token=token)
    nc.finalize()
    return nc


_NC_CACHE = {}


def _get_nc(mm_fast=True, repeat=1):
    key = (mm_fast, repeat)
    if key not in _NC_CACHE:
        _NC_CACHE[key] = build_nc(mm_fast, repeat)
    return _NC_CACHE[key]


def make_in_maps(x, y, W_qkv, W_out):
    x = np.asarray(x, np.float32).reshape(B, C, HW)
    y = np.asarray(y, np.float32).reshape(B, C, HW)
    W_qkv = np.asarray(W_qkv, np.float32)
    W_out = np.asarray(W_out, np.float32)
    in_maps = []
    for core in range(N_CORES):
        b, h = core // HEADS, core % HEADS
        sl = slice(h * D, (h + 1) * D)
        wqT = W_qkv[sl, :].T                                     # [C, D]
        wkT = W_qkv[HIDDEN + h * D:HIDDEN + (h + 1) * D, :].T
        wvT = W_qkv[2 * HIDDEN + h * D:2 * HIDDEN + (h + 1) * D, :].T
        bf = ml_dtypes.bfloat16
        in_maps.append({
            "xb": np.ascontiguousarray(x[b].astype(bf)),
            "yb": np.ascontiguousarray(y[b].astype(bf)),
            "wq2T": np.ascontiguousarray(
                np.concatenate([wqT, wqT], axis=1).astype(bf)),
            "wk2T": np.ascontiguousarray(
                np.concatenate([wkT, wkT], axis=1).astype(bf)),
            "wvT": np.ascontiguousarray(wvT.astype(bf)),
            "woT": np.ascontiguousarray(W_out[:, sl].T),
        })
    return in_maps


def gather(results, b_out):
    b_out = np.asarray(b_out, np.float32)
    out = np.zeros((B, C, HW), np.float32)
    for core in range(N_CORES):
        # per-head softmax-denominator division, folded to the host (it
        # commutes with the out-projection's channel contraction)
        out[core // HEADS] += (results[core]["outp"].astype(np.float32)
                               / results[core]["lout"])
    out += b_out[None, :, None]
    return out.reshape(B, C, 64, 64)


def kernel(x, y, W_qkv, W_out, b_out):
    nc = _get_nc(mm_fast=True)
    in_maps = make_in_maps(x, y, W_qkv, W_out)
    res = run_bass_kernel_spmd(nc, in_maps, core_ids=list(range(N_CORES)))
    return gather(res.results, b_out)


# revision 11
# speedup vs baseline: 2.9039x; 1.0207x over previous
"""Trainium2 Bass kernel for CrossAttention (B=2, C=128, H=W=64, heads=4, d=64).

Sharding: one (batch, head) pair per NeuronCore (2*4 = 8 cores).

v3 vs v2 (HW-ablation driven):
  - exp tiles are whole-[128,1024] Schraudolph, alternating ACT/DVE (each
    engine pays its ~300ns instruction overhead once per two pairs; HW A/B
    vs split-halves: ~138us vs ~146us).
  - QK operands are bf16 (q2/k2_r): bf16 moving streams measured 300ns vs
    396ns fp32r at N=512; precision impact ~5e-5 on logits.
  - outT/wo_r are bf16 (po matmul joins the fast path); the softmax
    denominator row goes to a separate f32 tile for the lout DMA.
"""

import os
import numpy as np
import ml_dtypes

import concourse.bacc as bacc
import concourse.mybir as mybir
from concourse.bass import ts, ds
from concourse.tile import TileContext
from concourse.bass_utils import run_bass_kernel_spmd

F32 = mybir.dt.float32
F32R = mybir.dt.float32r
BF16 = mybir.dt.bfloat16
I16 = mybir.dt.int16

B, C, HW = 2, 128, 4096
HEADS, D = 4, 64
HIDDEN = HEADS * D
SCALE = 10.0
N_CORES = 8

IC = 512             # i-axis chunk per pv accumulation
N_IC = HW // IC      # 8
N_J = HW // 128      # 32 j-tiles of 128
N_P = N_J // 2       # 16 j-pairs per i-chunk
TOT = N_IC * N_P     # 128 global pairs
NCH = HW // 512      # 8 projection chunks

LOOKAHEAD = 2
QK_BUFS = 3

LOG2E = 1.4426950408889634
EXP_A = SCALE * LOG2E * 128.0              # fold exp scale into Schraudolph mult
EXP_B = 127.0 * 128.0 - 60801.48 / 65536.0  # Schraudolph bias (min max-rel-err)


def _use_act(g):
    return g % 2 == 0


def _emit_one(nc, tc, io, mm_fast, rep, token=None):
    """Emit one full forward pass. rep distinguishes pool names across repeats."""
    xb, yb, wq2T, wk2T, wvT, woT, outp, lout = io
    MMDT = F32R if mm_fast else F32
    Square = mybir.ActivationFunctionType.Square
    Copy = mybir.ActivationFunctionType.Copy
    mult = mybir.AluOpType.mult
    add = mybir.AluOpType.add

    with tc.tile_pool(name=f"big{rep}", bufs=1) as big, \
         tc.tile_pool(name=f"const{rep}", bufs=1) as const:
        wq_sb = const.tile([C, 2 * D], BF16)
        wk_sb = const.tile([C, 2 * D], BF16)
        wv_sb = const.tile([C, D], BF16)
        wo_sb = const.tile([D, C], F32)
        wo_r = const.tile([D, C], BF16)
        x_r = big.tile([C, HW], BF16)
        y_r = big.tile([C, HW], BF16)
        if token is not None and rep > 0:
            # seed every input tile with the previous rep's token: the input
            # DMAs then carry a WAW dependency on it, serializing the reps
            for tile in (wq_sb, wk_sb, wv_sb, x_r, y_r):
                nc.vector.tensor_copy(tile[:, 0:1], token)
            nc.vector.tensor_copy(wo_sb[:, 0:1], token[0:D])
        # weights first: tiny DMAs that gate the first projection matmuls
        nc.sync.dma_start(wq_sb, wq2T[:])
        nc.sync.dma_start(wk_sb, wk2T[:])
        nc.sync.dma_start(wv_sb, wvT[:])
        nc.sync.dma_start(wo_sb, woT[:])

        # chunked input DMA so projections start before the full load lands;
        # xb on the SP HWDGE queue, yb on the ACT HWDGE queue (parallel)
        for t in range(NCH):
            nc.sync.dma_start(x_r[:, ts(t, 512)], xb[:, ts(t, 512)])
            nc.scalar.dma_start(y_r[:, ts(t, 512)], yb[:, ts(t, 512)])
        q2_sb = big.tile([C, HW], BF16)      # duplicated q (both halves)
        k2_sb = big.tile([C, HW], F32)       # duplicated k, pre-norm
        k2_r = big.tile([C, HW], BF16)       # duplicated k, norm folded
        vT_sb = big.tile([128, N_J, D + 1], BF16)
        qparts = const.tile([C, NCH], F32)
        kparts = const.tile([C, NCH], F32)
        ones_sb = const.tile([128, 1], F32)
        nc.vector.memset(ones_sb, 1.0)

        # ---------------- Stage A: projections + normalization ----------
        with tc.tile_pool(name=f"psA{rep}", bufs=2, space="PSUM") as psA:
            # PE warm-up: dense dummy matmuls during the DMA window nudge the
            # HAM clock gate toward 2.4 GHz before the real work lands.
            warm = const.tile([128, 512], F32)
            nc.vector.memset(warm, 0.0)
            for w in range(2):
                pw = psA.tile([128, 512], F32, tag="pw")
                nc.tensor.matmul(pw, lhsT=warm[:, 0:128], rhs=warm[:],
                                 start=True, stop=True)
            nc.vector.tensor_copy(wo_r[:], wo_sb[:])
            for t in range(NCH):
                pq = psA.tile([C, 512], F32, tag="pq")
                nc.tensor.matmul(pq, lhsT=wq_sb[:], rhs=x_r[:, ts(t, 512)],
                                 start=True, stop=True)
                # squares first so the norm-reduction chain unblocks early;
                # q copies alternate ACT/DVE to balance stage-A engine load
                scr_ps = psA.tile([128, 512], F32, tag="pw")
                nc.scalar.activation(scr_ps, pq, Square,
                                     accum_out=qparts[:, t:t + 1])
                if t % 2 == 0:
                    nc.scalar.copy(q2_sb[:, ts(t, 512)], pq)
                else:
                    nc.vector.tensor_copy(q2_sb[:, ts(t, 512)], pq)
                pk = psA.tile([C, 512], F32, tag="pk")
                nc.tensor.matmul(pk, lhsT=wk_sb[:], rhs=y_r[:, ts(t, 512)],
                                 start=True, stop=True)
                scr_ps2 = psA.tile([128, 512], F32, tag="pw")
                nc.scalar.activation(scr_ps2, pk, Square,
                                     accum_out=kparts[:, t:t + 1])
                nc.vector.tensor_copy(k2_sb[:, ts(t, 512)], pk)
                # vT blocks for this yb chunk: (yb 128-col).T @ wv -> [128, 64]
                # 4 blocks share one PSUM tile so a single DVE copy moves them
                pv = psA.tile([128, 4, D], F32, tag="pv")
                for jj in range(4):
                    nc.tensor.matmul(pv[:, jj, :],
                                     lhsT=y_r[:, ts(4 * t + jj, 128)],
                                     rhs=wv_sb[:], start=True, stop=True)
                nc.vector.tensor_copy(vT_sb[:, 4 * t:4 * t + 4, 0:D], pv)
            nc.vector.tensor_copy(vT_sb[:, :, D],
                                  ones_sb.to_broadcast((128, N_J)))

        # g[d] = 1/sqrt(ssq_q[d] * ssq_k[d]) on DVE: bit-trick rsqrt seed +
        # 2 Newton iterations, all at [128, 1] (duplicated rows give a
        # duplicated g for free).
        ssq_q = const.tile([C, 1], F32)
        ssq_k = const.tile([C, 1], F32)
        nc.vector.reduce_sum(ssq_q, qparts[:], axis=mybir.AxisListType.X)
        nc.vector.reduce_sum(ssq_k, kparts[:], axis=mybir.AxisListType.X)
        P = const.tile([C, 1], F32)
        nc.vector.tensor_mul(P, ssq_q, ssq_k)
        nc.vector.tensor_scalar_max(P, P, 1e-24)
        hi = const.tile([C, 1], mybir.dt.int32)
        # 0x5f3759df - h == ((h >> 1) ^ 0xffffffff) + 0x5f3759e0, fused
        nc.vector.tensor_scalar(hi, P.bitcast(mybir.dt.int32), 1, -1,
                                op0=mybir.AluOpType.arith_shift_right,
                                op1=mybir.AluOpType.bitwise_xor)
        nc.vector.tensor_scalar(hi, hi, 0x5F3759E0, None,
                                op0=mybir.AluOpType.add)
        g = const.tile([C, 1], F32)
        gt = const.tile([C, 1], F32)
        yv = hi.bitcast(F32)
        for it in range(2):
            src = yv if it == 0 else g
            nc.vector.tensor_mul(gt, src, src)                       # y^2
            nc.vector.scalar_tensor_tensor(gt, gt, -0.5, P,
                                           op0=mult, op1=mult)       # -.5Py^2
            nc.vector.scalar_tensor_tensor(g, gt, 1.5, src,
                                           op0=mybir.AluOpType.add,
                                           op1=mult)                 # refined
        # fold both norms into k; tiny first chunk so QK(0) unblocks early
        nc.vector.tensor_scalar(k2_r[:, 0:256], k2_sb[:, 0:256],
                                g, None, op0=mult)
        nc.vector.tensor_scalar(k2_r[:, 256:1024], k2_sb[:, 256:1024],
                                g, None, op0=mult)
        for c4 in range(1, 4):
            nc.vector.tensor_scalar(k2_r[:, ts(c4, 1024)], k2_sb[:, ts(c4, 1024)],
                                    g, None, op0=mult)

        # ------- Main loop: attention in j-pairs + fused epilogue ---------
        # Per global pair g = 16*ic + p covering j-tiles (2p, 2p+1) on
        # i-chunk ic (256 wide): QK row-tiled pair -> one exp tile
        # [128, 512] (ACT or DVE Schraudolph) -> 2 accumulating PV matmuls.
        with tc.tile_pool(name=f"qkps{rep}", bufs=QK_BUFS, space="PSUM") as qkps_pool, \
             tc.tile_pool(name=f"pvps{rep}", bufs=1, space="PSUM") as pvps_pool, \
             tc.tile_pool(name=f"pops{rep}", bufs=1, space="PSUM") as pops_pool, \
             tc.tile_pool(name=f"eta{rep}", bufs=5) as ea_pool, \
             tc.tile_pool(name=f"ot{rep}", bufs=2) as ot_pool:
            qk_tiles = {}
            e_tiles = {}
            pv_tiles = {}

            def emit_qk(g):
                ic, p = divmod(g, N_P)
                qk = qkps_pool.tile([128, 2 * IC], F32, tag="qk")
                oc = ds(ic * IC, IC)
                nc.tensor.matmul(qk[:, 0:IC],
                                 lhsT=k2_r[0:D, ts(2 * p, 128)],
                                 rhs=q2_sb[0:D, oc],
                                 start=True, stop=True,
                                 tile_position=(0, 0))
                nc.tensor.matmul(qk[:, IC:2 * IC],
                                 lhsT=k2_r[D:2 * D, ts(2 * p + 1, 128)],
                                 rhs=q2_sb[D:2 * D, oc],
                                 start=True, stop=True,
                                 tile_position=(64, 0))
                qk_tiles[g] = qk

            def emit_exp(g):
                qk = qk_tiles.pop(g)
                # Schraudolph bit-trick exp, whole tiles ALTERNATING between
                # ACT and DVE: each engine pays the ~300ns per-instruction
                # overhead (decode+PSUM access+sem) once per TWO pairs, which
                # beats the split-halves scheme's once-per-pair.
                eT = ea_pool.tile([128, 2 * IC], BF16, tag="eT")
                if _use_act(g):
                    nc.scalar.activation(eT.bitcast(I16), qk, Copy,
                                         bias=EXP_B, scale=EXP_A)
                else:
                    nc.vector.tensor_scalar(eT.bitcast(I16), qk, EXP_A, EXP_B,
                                            op0=mult, op1=add)
                e_tiles[g] = eT

            def emit_pv(g):
                ic, p = divmod(g, N_P)
                if p == 0:
                    pv_ps = pvps_pool.tile([D + 1, IC], F32, tag="pv")
                    pv_tiles[ic] = pv_ps
                eT = e_tiles.pop(g)
                nc.tensor.matmul(pv_tiles[ic][:], lhsT=vT_sb[:, 2 * p, :],
                                 rhs=eT[:, 0:IC],
                                 start=(p == 0), stop=False)
                nc.tensor.matmul(pv_tiles[ic][:], lhsT=vT_sb[:, 2 * p + 1, :],
                                 rhs=eT[:, IC:2 * IC],
                                 start=False, stop=(p == N_P - 1))

            def epilogue(ic):
                pv_ps = pv_tiles.pop(ic)
                oc = ds(ic * IC, IC)
                outT = ot_pool.tile([D, IC], BF16, tag="outT")
                lT = ot_pool.tile([1, IC], F32, tag="lT")
                # split the pv evacuation across both elementwise engines so
                # whichever frees first starts; pv_ps unblocks ~2x sooner
                nc.scalar.copy(outT[:, 0:IC // 2], pv_ps[0:D, 0:IC // 2])
                nc.vector.tensor_copy(outT[:, IC // 2:IC],
                                      pv_ps[0:D, IC // 2:IC])
                nc.vector.tensor_copy(lT, pv_ps[D:D + 1, :])
                po = pops_pool.tile([C, IC], F32, tag="po")
                nc.tensor.matmul(po, lhsT=wo_r[:], rhs=outT[:],
                                 start=True, stop=True)
                out_sb = ot_pool.tile([C, IC], BF16, tag="out_sb")
                nc.scalar.copy(out_sb[:, 0:IC // 2], po[:, 0:IC // 2])
                nc.vector.tensor_copy(out_sb[:, IC // 2:IC], po[:, IC // 2:IC])
                nc.sync.dma_start(outp[:, oc], out_sb)
                nc.sync.dma_start(lout[:, oc], lT)
                if token is not None and ic == N_IC - 1:
                    nc.vector.tensor_copy(token, out_sb[:, 0:1])

            for g in range(LOOKAHEAD):
                emit_qk(g)
            for g in range(TOT):
                if g + LOOKAHEAD < TOT:
                    emit_qk(g + LOOKAHEAD)
                emit_exp(g)
                emit_pv(g)
                if g % N_P == N_P - 1:
                    epilogue(g // N_P)


def build_nc(mm_fast=True, repeat=1, probe=False):
    nc = bacc.Bacc(None, target_bir_lowering=False)
    xb = nc.dram_tensor("xb", [C, HW], BF16, kind="ExternalInput")
    yb = nc.dram_tensor("yb", [C, HW], BF16, kind="ExternalInput")
    wq2T = nc.dram_tensor("wq2T", [C, 2 * D], BF16, kind="ExternalInput")
    wk2T = nc.dram_tensor("wk2T", [C, 2 * D], BF16, kind="ExternalInput")
    wvT = nc.dram_tensor("wvT", [C, D], BF16, kind="ExternalInput")
    woT = nc.dram_tensor("woT", [D, C], F32, kind="ExternalInput")
    outp = nc.dram_tensor("outp", [C, HW], BF16, kind="ExternalOutput")
    lout = nc.dram_tensor("lout", [1, HW], F32, kind="ExternalOutput")
    io = (xb, yb, wq2T, wk2T, wvT, woT, outp, lout)
    with TileContext(nc) as tc:
        with tc.tile_pool(name="tok", bufs=1) as tokp:
            token = (tokp.tile([C, 1], F32, name="token")
                     if repeat > 1 else None)
            for rep in range(repeat):
                _emit_one(nc, tc, io, mm_fast, rep, token=token)
    nc.finalize()
    return nc


_NC_CACHE = {}


def _get_nc(mm_fast=True, repeat=1):
    key = (mm_fast, repeat)
    if key not in _NC_CACHE:
        _NC_CACHE[key] = build_nc(mm_fast, repeat)
    return _NC_CACHE[key]


def make_in_maps(x, y, W_qkv, W_out):
    x = np.asarray(x, np.float32).reshape(B, C, HW)
    y = np.asarray(y, np.float32).reshape(B, C, HW)
    W_qkv = np.asarray(W_qkv, np.float32)
    W_out = np.asarray(W_out, np.float32)
    in_maps = []
    for core in range(N_CORES):
        b, h = core // HEADS, core % HEADS
        sl = slice(h * D, (h + 1) * D)
        wqT = W_qkv[sl, :].T                                     # [C, D]
        wkT = W_qkv[HIDDEN + h * D:HIDDEN + (h + 1) * D, :].T
        wvT = W_qkv[2 * HIDDEN + h * D:2 * HIDDEN + (h + 1) * D, :].T
        bf = ml_dtypes.bfloat16
        in_maps.append({
            "xb": np.ascontiguousarray(x[b].astype(bf)),
            "yb": np.ascontiguousarray(y[b].astype(bf)),
            "wq2T": np.ascontiguousarray(
                np.concatenate([wqT, wqT], axis=1).astype(bf)),
            "wk2T": np.ascontiguousarray(
                np.concatenate([wkT, wkT], axis=1).astype(bf)),
            "wvT": np.ascontiguousarray(wvT.astype(bf)),
            "woT": np.ascontiguousarray(W_out[:, sl].T),
        })
    return in_maps


def gather(results, b_out):
    b_out = np.asarray(b_out, np.float32)
    out = np.zeros((B, C, HW), np.float32)
    for core in range(N_CORES):
        # per-head softmax-denominator division, folded to the host (it
        # commutes with the out-projection's channel contraction)
        out[core // HEADS] += (results[core]["outp"].astype(np.float32)
                               / results[core]["lout"])
    out += b_out[None, :, None]
    return out.reshape(B, C, 64, 64)


def kernel(x, y, W_qkv, W_out, b_out):
    nc = _get_nc(mm_fast=True)
    in_maps = make_in_maps(x, y, W_qkv, W_out)
    res = run_bass_kernel_spmd(nc, in_maps, core_ids=list(range(N_CORES)))
    return gather(res.results, b_out)
